# revision 1
# baseline (speedup 1.0000x reference)
"""AffinityPropagate3 Trainium2 kernel, v3.

Per-core (one batch sample): K = softmax(guided, 0); mask = sign(sparse);
x_{t+1} = mask*x0 + (1-mask) * sum_k K_k * shift_k(x_t), 16 steps.

Layout: xa [120 part, 6 row-slots, 642 cols] fp16, double-buffered.
Row slots: 0 = top halo, 1-4 = rows 4p..4p+3, 5 = bottom halo; col 0/641 pad.
All 9 taps read xa directly (odd-element offsets keep DVE 2x mode -- measured).

Per iteration:
  DVE:  8 products (taps 3,5 and 1 split into row-pair halves for drain chase)
  Pool: 1 product (tap 4)
  PE:   m0 + 9 z_k identity-matmul accumulate into PSUM banks 0-4; plus two
        shifted-identity matmuls producing next iteration's halo rows in
        PSUM banks 5-7 (replaces all per-iteration DMA)
  ACT:  psum->xa drains (d1: rows 0-1, d2: rows 2-3) + 2 halo drains
"""

import sys

for _p in ("/opt/trn_rl_repo", "/root/.axon_site/_ro/trn_rl_repo"):
    if _p not in sys.path:
        sys.path.insert(0, _p)

import numpy as np

from concourse import bacc, mybir
from concourse import tile
from concourse.bass_utils import run_bass_kernel_spmd


def dedup_ldweights(nc):
    """Drop InstLdweights whose stationary matches the previous one (PE
    weights persist).  An Ldweights carrying sync info becomes a NoOp."""
    for f in nc.m.functions:
        for bb in f.blocks:
            out = []
            seen_key = None
            changed = False
            for ins in bb.instructions:
                if type(ins).__name__ == "InstLdweights":
                    key = str(ins.ins[0])
                    if key == seen_key:
                        si = ins.sync_info
                        if si is not None and (si.on_wait or si.on_update):
                            out.append(
                                mybir.InstNoOp(
                                    name=ins.name + "-ldwn",
                                    engine=ins.engine,
                                    sync_info=si,
                                )
                            )
                        changed = True
                        continue
                    seen_key = key
                out.append(ins)
            if changed:
                bb.instructions[:] = out


B = 8
H, W = 480, 640
P = 120
RPP = 4
WP = W + 2
NJ = RPP + 2
FLAT = RPP * W       # 2560
CHUNK = 512
NCH = FLAT // CHUNK  # 5
HFLAT = FLAT // 2    # 1280
PROP_TIME = 16

FP32 = mybir.dt.float32
FP16 = mybir.dt.float16

TAPS = [(ki - 1, kj - 1) for ki in range(3) for kj in range(3)]
POOL_TAP = 4


def _rows_view(dram_ap):
    return dram_ap.rearrange("(p r) w -> p r w", p=P)


def build_program(compile_=True):
    nc = bacc.Bacc("TRN2", target_bir_lowering=False, debug=False, num_devices=B)

    guided_d = nc.dram_tensor("guided", [9, H, W], FP32, kind="ExternalInput")
    x_d = nc.dram_tensor("x", [H, W], FP32, kind="ExternalInput")
    sparse_d = nc.dram_tensor("sparse_depth", [H, W], FP32, kind="ExternalInput")
    out_d = nc.dram_tensor("out", [H, W], FP32, kind="ExternalOutput")

    ident_d = nc.inline_tensor(np.eye(P, dtype=np.float16), name="ident_const")
    # shift_up: out[i] = rhs[i-1] (top halo);  shift_dn: out[i] = rhs[i+1]
    su_d = nc.inline_tensor(np.eye(P, k=1, dtype=np.float16), name="shift_up")
    sd_d = nc.inline_tensor(np.eye(P, k=-1, dtype=np.float16), name="shift_dn")

    dma_engines = [nc.sync, nc.scalar, nc.gpsimd]

    with tile.TileContext(nc) as tc:
        with (
            tc.tile_pool(name="persist", bufs=1) as persist,
            tc.tile_pool(name="work32", bufs=3) as work32,
            tc.tile_pool(name="psum", bufs=1, space="PSUM") as psump,
        ):
            xa = [
                persist.tile([P, NJ, WP], FP16, tag=f"xa{i}", name=f"xa{i}")
                for i in range(2)
            ]
            wk = [
                persist.tile([P, FLAT], FP16, tag=f"wk{k}", name=f"wk{k}")
                for k in range(9)
            ]
            z = [
                persist.tile([P, RPP, W], FP16, tag=f"z{k}", name=f"z{k}")
                for k in range(9)
            ]
            m0 = persist.tile([P, FLAT], FP16, tag="m0")
            nomask = persist.tile([P, FLAT], FP16, tag="nomask")
            rf16 = persist.tile([P, FLAT], FP16, tag="rf16")
            ident = persist.tile([P, P], FP16, tag="ident")
            shup = persist.tile([P, P], FP16, tag="shup")
            shdn = persist.tile([P, P], FP16, tag="shdn")
            stag = persist.tile([P, RPP, W], FP32, tag="stag")

            psum = psump.tile([P, FLAT], FP32, tag="psum")       # banks 0-4
            psum_h = psump.tile([P, 1408], FP32, tag="psum_h")   # banks 5-7
            # psum_h[0:640]   = top-halo row content (cols 1..640)
            # psum_h[768:1408] = bottom-halo row content (cols 1..640)

            # ---- init ----
            nc.vector.memset(xa[0][:], 0.0)
            nc.gpsimd.memset(xa[1][:], 0.0)
            nc.sync.dma_start(out=ident[:], in_=ident_d[:])
            nc.sync.dma_start(out=shup[:], in_=su_d[:])
            nc.sync.dma_start(out=shdn[:], in_=sd_d[:])

            # ---- x load (cast fp32->fp16 via SWDGE) + initial halos ----
            xd = _rows_view(x_d[:])
            nc.gpsimd.dma_start(out=xa[0][:, 1 : 1 + RPP, 1 : 1 + W], in_=xd)
            nc.gpsimd.dma_start(
                out=xa[0][1:P, 0:1, 1 : 1 + W], in_=xd[0 : P - 1, 3:4, :]
            )
            nc.gpsimd.dma_start(
                out=xa[0][0 : P - 1, 5:6, 1 : 1 + W], in_=xd[1:P, 0:1, :]
            )

            # ---- masks ----
            sp = work32.tile([P, RPP, W], FP32, tag="g32", name="sp32")
            nc.sync.dma_start(out=sp[:], in_=_rows_view(sparse_d[:]))
            nc.vector.tensor_scalar(
                out=nomask[:], in0=sp.rearrange("p a b -> p (a b)")[:],
                scalar1=0.0, scalar2=None, op0=mybir.AluOpType.is_equal,
            )
            xv = xa[0][:, 1 : 1 + RPP, 1 : 1 + W]
            m0v = m0.rearrange("p (a b) -> p a b", a=RPP)
            nc.vector.tensor_tensor(
                out=m0v[:], in0=nomask.rearrange("p (a b) -> p a b", a=RPP)[:],
                in1=xv, op=mybir.AluOpType.mult,
            )
            nc.vector.tensor_tensor(
                out=m0v[:], in0=xv, in1=m0v[:], op=mybir.AluOpType.subtract
            )

            # ---- guided loads (3-engine spread) + exp; iteration 1 runs on
            # UNNORMALIZED weights, chasing each channel's DMA inside the
            # load wall: z_k = exp(g_k) * tap_k(x0) accumulates on PE while
            # the denominator accumulates on DVE.  Normalization is applied
            # once at drain time: x1 = m0 + (nomask/den) * psum. ----
            wkv0 = [wk[k].rearrange("p (a b) -> p a b", a=RPP) for k in range(9)]
            den16 = persist.tile([P, FLAT], FP16, tag="den16")
            for k in range(9):
                g32 = work32.tile([P, RPP, W], FP32, tag="g32", name=f"g32_{k}")
                dma_engines[k % 3].dma_start(out=g32[:], in_=_rows_view(guided_d[k]))
                nc.scalar.activation(
                    out=wk[k][:], in_=g32.rearrange("p a b -> p (a b)")[:],
                    func=mybir.ActivationFunctionType.Exp,
                )
                dh, dw = TAPS[k]
                nc.vector.tensor_tensor(
                    out=z[k][:], in0=wkv0[k][:],
                    in1=xa[0][:, 1 + dh : 1 + dh + RPP, 1 + dw : 1 + dw + W],
                    op=mybir.AluOpType.mult,
                )
                if k == 0:
                    nc.vector.tensor_copy(out=den16[:], in_=wk[0][:])
                else:
                    nc.vector.tensor_tensor(
                        out=den16[:], in0=den16[:], in1=wk[k][:],
                        op=mybir.AluOpType.add,
                    )
                zk = z[k].rearrange("p a b -> p (a b)")
                for c in range(NCH):
                    nc.tensor.matmul(
                        out=psum[:, c * CHUNK : (c + 1) * CHUNK],
                        lhsT=ident[:],
                        rhs=zk[:, c * CHUNK : (c + 1) * CHUNK],
                        start=(k == 0),
                        stop=(k == 8),
                    )

            # ---- normalization: rf16 = nomask / sum_k exp ----
            den32 = work32.tile([P, FLAT], FP32, tag="g32", name="den32")
            nc.vector.tensor_copy(out=den32[:], in_=den16[:])
            r32 = work32.tile([P, FLAT], FP32, tag="g32", name="r32")
            nc.vector.reciprocal_approx_fast(out=r32[:], in_=den32[:])
            nc.vector.tensor_tensor(
                out=rf16[:], in0=r32[:], in1=nomask[:], op=mybir.AluOpType.mult
            )

            def scale_wk(k):
                nc.vector.tensor_tensor(
                    out=wk[k][:], in0=wk[k][:], in1=rf16[:],
                    op=mybir.AluOpType.mult,
                )

            # helpers -------------------------------------------------------
            def wkv(k):
                return wk[k].rearrange("p (a b) -> p a b", a=RPP)

            def tap_view(xsrc, k, half=None):
                dh, dw = TAPS[k]
                r0 = 1 + dh
                c0 = 1 + dw
                if half is None:
                    return xsrc[:, r0 : r0 + RPP, c0 : c0 + W]
                if half == 0:
                    return xsrc[:, r0 : r0 + 2, c0 : c0 + W]
                return xsrc[:, r0 + 2 : r0 + RPP, c0 : c0 + W]

            def dve_tap(xsrc, k, half=None):
                if half is None:
                    o, i0 = z[k][:], wkv(k)[:]
                elif half == 0:
                    o, i0 = z[k][:, 0:2], wkv(k)[:, 0:2]
                else:
                    o, i0 = z[k][:, 2:4], wkv(k)[:, 2:4]
                nc.vector.tensor_tensor(
                    out=o, in0=i0, in1=tap_view(xsrc, k, half),
                    op=mybir.AluOpType.mult,
                )

            def pe_pass(src_flat, chunks, start, stops):
                """matmul src_flat over [lo,hi) chunks; stops: set per chunk."""
                for lo, hi in chunks:
                    nc.tensor.matmul(
                        out=psum[:, lo:hi],
                        lhsT=ident[:],
                        rhs=src_flat[:, lo:hi],
                        start=start,
                        stop=(lo, hi) in stops,
                    )

            FIVE = [(c * CHUNK, (c + 1) * CHUNK) for c in range(NCH)]
            A3 = [(0, 512), (512, 1024), (1024, 1280)]          # flat rows 0-1
            B3 = [(1280, 1536), (1536, 2048), (2048, 2560)]     # flat rows 2-3
            STOP_A = {(0, 512), (512, 1024)}
            STOP_B = {(1280, 1536), (1536, 2048), (2048, 2560)}

            # ---- 16 propagation iterations (iteration 0 = the raw pass
            # emitted above; only its drain + tail are emitted here) ----
            rfv = rf16.rearrange("p (a b) -> p a b", a=RPP)
            for t in range(PROP_TIME):
                xar = xa[t % 2]
                xaw = xa[1 - t % 2]
                last = t == PROP_TIME - 1
                first = t == 1

                # All products on DVE, ordered by gate:
                # d2xa(t-1)-gated halves first, then d1xa-gated, then
                # halo-dependent fulls, z1a (top halo) last.
                if t > 0:
                    for k, half in (
                        (3, 1), (5, 1), (4, 1), (1, 1),
                        (3, 0), (5, 0), (4, 0),
                        (0, None), (2, None), (6, None), (8, None), (7, None),
                        (1, 0),
                    ):
                        if first and half in (1, None):
                            scale_wk(k)
                        dve_tap(xar, k, half)

                    # PE accumulation; m0 pass was woven into the previous
                    # iteration's tail.
                    zf = [z[k].rearrange("p a b -> p (a b)") for k in range(9)]
                    pe_pass(zf[3], B3, False, set())
                    pe_pass(zf[5], B3, False, set())
                    pe_pass(zf[4], B3, False, set())
                    pe_pass(zf[1], B3, False, set())
                    pe_pass(zf[3], A3, False, set())
                    pe_pass(zf[5], A3, False, set())
                    pe_pass(zf[4], A3, False, set())
                    pe_pass(zf[0], FIVE, False, set())
                    pe_pass(zf[2], FIVE, False, set())
                    pe_pass(zf[6], FIVE, False, set())
                    pe_pass(zf[8], FIVE, False, set())
                    # z7 closes the B banks; z1's A half closes the A banks
                    pe_pass(
                        zf[7], FIVE, False,
                        {(1536, 2048), (2048, 2560)},
                    )
                    pe_pass(zf[1], A3, False, STOP_A | {(1024, 1280)})

                pv = psum.rearrange("p (a b) -> p a b", a=RPP)
                if not last:
                    # d2xa first (closed by z7 pass), then d1xa (z1a pass);
                    # iteration 0 normalizes at drain time on DVE:
                    # x1 = m0 + rf16 * psum_raw
                    if t == 0:
                        nc.vector.tensor_tensor(
                            out=z[0][:, 2:4], in0=rfv[:, 2:4], in1=pv[:, 2:4],
                            op=mybir.AluOpType.mult,
                        )
                        nc.vector.tensor_tensor(
                            out=xaw[:, 3:5, 1 : 1 + W], in0=z[0][:, 2:4],
                            in1=m0v[:, 2:4], op=mybir.AluOpType.add,
                        )
                    else:
                        nc.scalar.copy(out=xaw[:, 3:5, 1 : 1 + W], in_=pv[:, 2:4])
                    # next iteration's m0 chunks 3-4 + top-halo matmuls
                    for lo, hi in ((1536, 2048), (2048, 2560)):
                        nc.tensor.matmul(
                            out=psum[:, lo:hi], lhsT=ident[:],
                            rhs=m0[:, lo:hi], start=True, stop=False,
                        )
                    nc.tensor.matmul(
                        out=psum_h[:, 0:512], lhsT=shup[:],
                        rhs=xaw[:, 4:5, 1:513], start=True, stop=True,
                    )
                    nc.tensor.matmul(
                        out=psum_h[:, 512:640], lhsT=shup[:],
                        rhs=xaw[:, 4:5, 513:641], start=True, stop=True,
                    )
                    if t == 0:
                        nc.vector.tensor_tensor(
                            out=z[0][:, 0:2], in0=rfv[:, 0:2], in1=pv[:, 0:2],
                            op=mybir.AluOpType.mult,
                        )
                        nc.vector.tensor_tensor(
                            out=xaw[:, 1:3, 1 : 1 + W], in0=z[0][:, 0:2],
                            in1=m0v[:, 0:2], op=mybir.AluOpType.add,
                        )
                    else:
                        nc.scalar.copy(out=xaw[:, 1:3, 1 : 1 + W], in_=pv[:, 0:2])
                    # m0 chunks 0-2 (c2 also overlaps d1xa's read)
                    for lo, hi in ((0, 512), (512, 1024), (1024, 1536)):
                        nc.tensor.matmul(
                            out=psum[:, lo:hi], lhsT=ident[:],
                            rhs=m0[:, lo:hi], start=True, stop=False,
                        )
                    # bottom-halo matmuls: slot5 <- row 0 (slot 1) of p+1
                    nc.tensor.matmul(
                        out=psum_h[:, 768:1024], lhsT=shdn[:],
                        rhs=xaw[:, 1:2, 1:257], start=True, stop=True,
                    )
                    nc.tensor.matmul(
                        out=psum_h[:, 1024:1408], lhsT=shdn[:],
                        rhs=xaw[:, 1:2, 257:641], start=True, stop=True,
                    )
                    # halo drains (ACT): top first (feeds z0/z2 early)
                    nc.scalar.copy(
                        out=xaw[:, 0:1, 1 : 1 + W], in_=psum_h[:, 0:640]
                    )
                    nc.scalar.copy(
                        out=xaw[:, 5:6, 1 : 1 + W], in_=psum_h[:, 768:1408]
                    )
                else:
                    # drain + stream out each half as soon as it closes
                    odv = _rows_view(out_d[:])
                    nc.scalar.copy(out=stag[:, 2:4], in_=pv[:, 2:4])
                    nc.sync.dma_start(out=odv[:, 2:4], in_=stag[:, 2:4])
                    nc.scalar.copy(out=stag[:, 0:2], in_=pv[:, 0:2])
                    nc.sync.dma_start(out=odv[:, 0:2], in_=stag[:, 0:2])

    dedup_ldweights(nc)
    if compile_:
        nc.compile()
    return nc


_CACHED_NC = None


def _get_nc():
    global _CACHED_NC
    if _CACHED_NC is None:
        _CACHED_NC = build_program()
    return _CACHED_NC


def kernel(guided, x, sparse_depth, _trace=False, _trace_kwargs=None):
    guided = np.ascontiguousarray(guided, dtype=np.float32)
    x = np.ascontiguousarray(x, dtype=np.float32)
    sparse_depth = np.ascontiguousarray(sparse_depth, dtype=np.float32)
    assert guided.shape == (B, 9, H, W)

    nc = _get_nc()
    in_maps = [
        {
            "guided": guided[b],
            "x": x[b, 0],
            "sparse_depth": sparse_depth[b, 0],
        }
        for b in range(B)
    ]
    res = run_bass_kernel_spmd(
        nc, in_maps, list(range(B)), trace=_trace, **(_trace_kwargs or {})
    )
    out = np.stack([res.results[b]["out"] for b in range(B)])[:, None]
    if _trace:
        return out.astype(np.float32), res
    return out.astype(np.float32)

